# revision 20
# baseline (speedup 1.0000x reference)
"""Trainium2 Bass kernel for CrossModalAttention — v3.

Reference (B=1, C=64, N=8192): two cross-attention directions (CT queries
over MRI K/V and vice versa), each an 8192x8192 softmax attention, fused
output projection.

Sharding: each of 8 cores owns 1024 queries for BOTH directions; K/V span
the full sequence. The host precomputes everything that is per-problem
cheap (Q/K projection composed into one 65x65 matrix applied to the query
features, the V projection in fp32, the DoubleRow-interleaved fp8 V^T
layout), so the device does ONLY the O(N^2) work: scores matmul -> exp ->
AV matmul.  The per-query softmax normalization and the final output
projection run on the host from the returned unnormalized accumulators
(65th row = denominator via a ones-column in V^T): normalization is
per-QUERY (per PSUM column), which no vector engine can broadcast across
partitions, and shipping it out costs only a 2KB DMA per block.

Device inner loop per direction and 512-query block, per 256-key group
jg (AV trails scores by LAG=3 groups so the PE never waits on exp):
  scoresT (128j, 2x512i per block) = matmul(lhsT=feat_aug fp16 (65,128),
                                            rhs=qq fp16 (65,512))  x2 j-sub
  exp: even jg (and jg=15) on ScalarE (ACTIVATE Exp, PSUM f32 -> SBUF
       fp8e4), odd jg on DVE (Schraudolph bit-trick: the fp8e4 bit
       pattern of ~exp(s) is round(s*8*log2e + 55.5) -- the HW DVE
       f32->int8 convert rounds-to-nearest, hence 55.5 not 56; one
       TENSOR_SCALAR op).  Splitting exp across BOTH engines halves the
       softmax bottleneck (16.8M exp/core: 145us on ScalarE alone in the
       v1 kernel); ScalarE takes 17/32 groups (it is slightly faster).
  AV:  att_b (65, 512) += DoubleRow matmul(lhsT=vT fp8 (128,[2,65] @80B),
                                           rhs=exp fp8 (128,[2,512]))
       fp8 DoubleRow contracts 256 keys per pass (2 MACs/cell/cycle) and
       streams the exp tile as the MOVING operand, so there are no
       per-chunk 128-col stationary reloads of exp tiles (which dominated
       PE time in v1).

DMA: two HWDGE queues — sync carries dir-0 tensors (first-use order:
half of qq0, first feature subtile, V^T in quarters, the rest) + the acc
outputs; the otherwise-idle gpsimd queue prefetches all dir-1 tensors in
parallel.

Accuracy: scores in fp16 (exact-ish), exp/V in fp8e4 (~3% quantization),
softmax renormalizes shared scale errors away; numpy-simulated end-to-end
rel err ~3-5e-3 (HW 6.4e-3) vs the 2e-2 gate.
"""

from contextlib import ExitStack

import numpy as np
import ml_dtypes

import concourse.bass as bass
import concourse.mybir as mybir
import concourse.tile as tile
from concourse import bacc
from concourse.bass_utils import run_bass_kernel_spmd

F32 = mybir.dt.float32
F16 = mybir.dt.float16
I8 = mybir.dt.int8
F8 = mybir.dt.float8e4

C = 64          # channels
N = 8192        # voxels (8*32*32)
NCORES = 8
NQ = N // NCORES      # 1024 queries per core
IH = 512              # query block (PSUM bank width in f32)
NIH = NQ // IH        # 2
W = C + 1             # 65: augmented channel dim
JG = 256              # keys per group (DoubleRow contracts 2x128)
NJG = N // JG         # 32 groups
VP = 80               # V^T bytes per (group, half): 65 padded to 16B align
LAG = 3               # groups the AV matmuls trail the score matmuls by

LOG2E = 1.4426950408889634
SCHRAUD_K1 = 8.0 * LOG2E       # fp8e4 has 3 mantissa bits
SCHRAUD_K2 = 55.5              # 7 (exp bias) * 8; tune +-0.5 for HW rounding

# feature subtile split (cols): small first tile so compute starts early
FSPLIT = [512, 512, 1024, 1024, 1024, 1024, 1024, 1024, 1024]
FOFF = np.cumsum([0] + FSPLIT).tolist()


def _emit_feat_load(eng, featp, feat_dram, name):
    subs = []
    for s, w in enumerate(FSPLIT):
        t = featp.tile([W, w], F16, tag="feat", name=f"{name}{s}")
        eng.dma_start(t[:], feat_dram[:, FOFF[s] : FOFF[s] + w])
        subs.append(t)
    return subs


def _feat_chunk(fs, j0):
    """AP of feat columns [j0, j0+128) from the split subtiles."""
    for s, w in enumerate(FSPLIT):
        if FOFF[s] <= j0 < FOFF[s + 1]:
            assert j0 + 128 <= FOFF[s + 1]
            return fs[s][:, j0 - FOFF[s] : j0 - FOFF[s] + 128]
    raise AssertionError(j0)


def _emit_attention(nc, pools, fs, qq, vt, acc, d):
    sp, ap, ep, cp = pools["sp"], pools["ap"], pools["ep"], pools["cp"]

    def emit_av(att, jg, et):
        nc.tensor.matmul(
            att[:],
            lhsT=vt[:]
            .bitcast(F8)
            .rearrange("p (jg two c) -> p jg two c", jg=NJG, two=2)[
                :, jg, :, :W
            ],
            rhs=et[:].bitcast(F8).rearrange("p (two n) -> p two n", two=2),
            perf_mode=mybir.MatmulPerfMode.DoubleRow,
            start=(jg == 0),
            stop=(jg == NJG - 1),
            skip_group_check=True,
        )

    for ih in range(NIH):
        att = ap.tile([W, IH], F32, tag="att", name=f"att{d}{ih}")
        pending = []
        for jg in range(NJG):
            ps = sp.tile([128, 2 * IH], F32, tag="ps", name=f"ps{d}{ih}{jg}")
            for h in range(2):
                j0 = JG * jg + 128 * h
                nc.tensor.matmul(
                    ps[:, IH * h : IH * (h + 1)],
                    lhsT=_feat_chunk(fs, j0),
                    rhs=qq[:, IH * ih : IH * (ih + 1)],
                    start=True,
                    stop=True,
                )
            et = ep.tile([128, 2 * IH], I8, tag="exp", name=f"et{d}{ih}{jg}")
            if jg % 2 == 0 or jg == 15:
                nc.scalar.activation(
                    et[:].bitcast(F8), ps[:], mybir.ActivationFunctionType.Exp
                )
            else:
                nc.vector.tensor_scalar(
                    et[:],
                    ps[:],
                    SCHRAUD_K1,
                    SCHRAUD_K2,
                    mybir.AluOpType.mult,
                    mybir.AluOpType.add,
                )
            pending.append((att, jg, et))
            if len(pending) > LAG:
                emit_av(*pending.pop(0))
        for args in pending:
            emit_av(*args)
        # unnormalized accumulators (+ row 64 = softmax denominator) out
        ot = cp.tile([W, IH], F32, tag="ot", name=f"ot{d}{ih}")
        if (d + ih) % 2 == 0:
            nc.vector.tensor_copy(ot[:], att[:])
        else:
            nc.scalar.copy(ot[:], att[:])
        nc.sync.dma_start(acc[:, NQ * d + IH * ih : NQ * d + IH * (ih + 1)], ot[:])


def _build_program(ctx, tc, feat0, feat1, qq0, qq1, vt0, vt1, acc):
    nc = tc.nc
    featp = ctx.enter_context(tc.tile_pool(name="feat", bufs=2 * len(FSPLIT)))
    pools = {
        "qp": ctx.enter_context(tc.tile_pool(name="qp", bufs=2)),
        "vp": ctx.enter_context(tc.tile_pool(name="vp", bufs=2)),
        "ep": ctx.enter_context(tc.tile_pool(name="ep", bufs=5)),
        "cp": ctx.enter_context(tc.tile_pool(name="cp", bufs=2)),
        "sp": ctx.enter_context(tc.tile_pool(name="spsum", bufs=3, space="PSUM")),
        "ap": ctx.enter_context(tc.tile_pool(name="apsum", bufs=2, space="PSUM")),
    }

    # sync queue: dir-0 tensors in first-use order; gpsimd queue: dir-1
    # prefetch in parallel (it is otherwise idle)
    qq0_sb = pools["qp"].tile([W, NQ], F16, tag="qq", name="qq0")
    nc.sync.dma_start(qq0_sb[:, :IH], qq0[:, :IH])
    fs0 = []
    vt0_sb = pools["vp"].tile([128, NJG * 2 * VP], I8, tag="vt", name="vt0")
    for s, w in enumerate(FSPLIT):
        t = featp.tile([W, w], F16, tag="feat", name=f"f0_{s}")
        nc.sync.dma_start(t[:], feat0[:, FOFF[s] : FOFF[s] + w])
        fs0.append(t)
        if s == 0:
            nc.sync.dma_start(qq0_sb[:, IH:], qq0[:, IH:])
        if s < 4:  # interleave V^T quarters so the first AV isn't starved
            q = NJG * 2 * VP // 4
            nc.sync.dma_start(
                vt0_sb[:, q * s : q * (s + 1)], vt0[:, q * s : q * (s + 1)]
            )

    qq1_sb = pools["qp"].tile([W, NQ], F16, tag="qq", name="qq1")
    nc.sync.dma_start(qq1_sb[:], qq1[:])
    fs1 = _emit_feat_load(nc.sync, featp, feat1, "f1_")
    vt1_sb = pools["vp"].tile([128, NJG * 2 * VP], I8, tag="vt", name="vt1")
    nc.sync.dma_start(vt1_sb[:], vt1[:])

    scratch = [
        nc.dram_tensor(f"scr{k}", [W, 64], F16, kind="Internal").ap()
        for k in range(3)
    ]
    holep = ctx.enter_context(tc.tile_pool(name="hole", bufs=2))

    _emit_attention(nc, pools, fs0, qq0_sb, vt0_sb, acc, 0)

    # deliberate ~6us PE-idle window at the direction boundary: a serial
    # DMA round-trip chain (FIFO sync queue places it after d0's last acc
    # write), ending with a 1-col rewrite of qq1 that gates d1's first
    # scores matmul.  The PE clock arbiter un-throttles within ~3us of PE
    # load dropping (observed in the ham trace) and has been seen to hold
    # K=8/8 through 100+us of subsequent full load -- trading ~6us idle
    # for direction 1 at 2.4 GHz instead of 1.2.
    prev = qq0_sb
    for k in range(3):
        nc.sync.dma_start(scratch[k][:], prev[:, :64])
        t = holep.tile([W, 64], F16, tag="hole", name=f"hole{k}")
        nc.sync.dma_start(t[:], scratch[k][:])
        prev = t
    nc.sync.dma_start(qq1_sb[:, 0:1], qq1[:, 0:1])

    _emit_attention(nc, pools, fs1, qq1_sb, vt1_sb, acc, 1)


def build_bass():
    nc = bacc.Bacc("TRN2", target_bir_lowering=False, debug=False)
    feat0 = nc.dram_tensor("feat0", [W, N], F16, kind="ExternalInput").ap()
    feat1 = nc.dram_tensor("feat1", [W, N], F16, kind="ExternalInput").ap()
    qq0 = nc.dram_tensor("qq0", [W, NQ], F16, kind="ExternalInput").ap()
    qq1 = nc.dram_tensor("qq1", [W, NQ], F16, kind="ExternalInput").ap()
    vt0 = nc.dram_tensor("vt0", [128, NJG * 2 * VP], I8, kind="ExternalInput").ap()
    vt1 = nc.dram_tensor("vt1", [128, NJG * 2 * VP], I8, kind="ExternalInput").ap()
    acc = nc.dram_tensor("acc", [W, 2 * NQ], F32, kind="ExternalOutput").ap()

    with tile.TileContext(nc) as tc, ExitStack() as ctx:
        _build_program(ctx, tc, feat0, feat1, qq0, qq1, vt0, vt1, acc)
    nc.compile()
    return nc


def _aug(w, b):
    # (out,in) weight + (out,) bias -> [w.T; b] of shape (in+1, out)
    return np.concatenate(
        [np.asarray(w, np.float32).T, np.asarray(b, np.float32)[None, :]], axis=0
    )


def _wkb(w, b):
    # (64, 65): [wk | bk] -- K projection folded onto the query side
    return np.concatenate(
        [np.asarray(w, np.float32), np.asarray(b, np.float32)[:, None]], axis=1
    )


def prepare_inputs(inputs):
    scale = np.float32(1.0 / np.sqrt(C))
    ct = np.asarray(inputs["ct_features"], np.float32).reshape(C, N)
    mri = np.asarray(inputs["mri_features"], np.float32).reshape(C, N)
    ones = np.ones((1, N), np.float32)
    ct_aug = np.concatenate([ct, ones], axis=0)
    mri_aug = np.concatenate([mri, ones], axis=0)

    # scores s[i,j] = (Wq q_i + bq) . (Wk k_j + bk) * scale
    #              = qq[:, i] . feat_aug[:, j],  qq = (Wq_aug @ [Wk|bk])^T @ q_aug
    wqq0 = _aug(
        np.asarray(inputs["wq_ct"]) * scale, np.asarray(inputs["bq_ct"]) * scale
    ) @ _wkb(inputs["wk_mri"], inputs["bk_mri"])
    wqq1 = _aug(
        np.asarray(inputs["wq_mri"]) * scale, np.asarray(inputs["bq_mri"]) * scale
    ) @ _wkb(inputs["wk_ct"], inputs["bk_ct"])
    qq0_full = (wqq0.T @ ct_aug).astype(np.float16)   # (65, N)
    qq1_full = (wqq1.T @ mri_aug).astype(np.float16)

    def vt_pack(w, b, feat_aug):
        # v_aug (65, N): V projection + ones row (softmax denominator), then
        # DoubleRow layout (128, NJG*2*VP): [p, jg*160 + h*80 + c] =
        # v_aug[c, 256*jg + 128*h + p], as fp8e4 bits viewed int8
        v = _aug(w, b).T @ feat_aug                     # (64, N)
        v_aug = np.concatenate([v, np.ones((1, N), np.float32)], axis=0)
        x = v_aug.reshape(W, NJG, 2, 128)               # (c, jg, h, p)
        x = x.transpose(3, 1, 2, 0)                     # (p, jg, h, c)
        xp = np.zeros((128, NJG, 2, VP), np.float32)    # pad c to 16B align
        xp[:, :, :, :W] = x
        xp = np.ascontiguousarray(xp.reshape(128, NJG * 2 * VP))
        return xp.astype(ml_dtypes.float8_e4m3).view(np.int8)

    vt0 = vt_pack(inputs["wv_mri"], inputs["bv_mri"], mri_aug)
    vt1 = vt_pack(inputs["wv_ct"], inputs["bv_ct"], ct_aug)
    feat0 = mri_aug.astype(np.float16)
    feat1 = ct_aug.astype(np.float16)

    in_maps = []
    for i in range(NCORES):
        sl = slice(NQ * i, NQ * (i + 1))
        in_maps.append(
            {
                "feat0": feat0,
                "feat1": feat1,
                "qq0": np.ascontiguousarray(qq0_full[:, sl]),
                "qq1": np.ascontiguousarray(qq1_full[:, sl]),
                "vt0": vt0,
                "vt1": vt1,
            }
        )
    return in_maps


def assemble_output(results, inputs):
    # acc rows 0..63 = sum_j exp * v (unnormalized), row 64 = sum_j exp
    acc = np.concatenate(
        [results[i]["acc"].reshape(W, 2, NQ) for i in range(NCORES)], axis=2
    )  # (65, 2, N)
    att = acc[:C] / acc[C:C + 1]  # (64, 2, N)
    fused = np.concatenate([att[:, 0], att[:, 1]], axis=0)  # (128, N)
    wo = np.asarray(inputs["wo"], np.float32)
    bo = np.asarray(inputs["bo"], np.float32)
    out = wo @ fused + bo[:, None]
    return out.reshape(1, C, 8, 32, 32).astype(np.float32)


_NC_CACHE = None


def _get_nc():
    global _NC_CACHE
    if _NC_CACHE is None:
        _NC_CACHE = build_bass()
    return _NC_CACHE


def kernel(**inputs):
    nc = _get_nc()
    in_maps = prepare_inputs(inputs)
    res = run_bass_kernel_spmd(nc, in_maps, list(range(NCORES)))
    return assemble_output(res.results, inputs)


if __name__ == "__main__":
    nc = build_bass()
    print("built OK")
